# revision 8
# baseline (speedup 1.0000x reference)
"""MQA attention with T5 relative-position bias on 8 Trainium2 NeuronCores.

Problem: x[2,2048,1024] -> q = x@Wq (16 heads x 64), shared single-head
k,v = x@Wkv; sim = q k^T * d^-0.5 + T5 bias; causal softmax; out @ Wout + bout.

Sharding: tensor-parallel over heads, 2 heads per core. Each core:
  - projects q for its 2 heads (scale folded into Wq) and the shared k,v
  - attention in simT layout ([j,i], j on partitions) so attn@V needs no
    transposes; softmax denominator rides as a ones-column in the V matmul
  - T5 bias: bias[d] is constant (rel_emb[31]) for distance d>=113; a
    constant-per-row offset is softmax-invariant, so only the near-diagonal
    band matters. It is applied as a multiply by host-precomputed
    E = exp((rel_emb[bucket(d)] - rel_emb[31]) * sqrt(64)) patterns (5 distinct
    [128,512] tiles per head); causal masking is folded in as E=0.
  - no row-max subtraction: logits are ~N(0,1) (max ~6.5), exp is safe in fp16
  - row-parallel out-projection -> per-core partial y; host sums partials+bout
"""

import numpy as np
import ml_dtypes

import concourse.tile as tile
from concourse import bacc, mybir
from concourse.bass_utils import run_bass_kernel_spmd

# ---- problem constants (hardcoded per contest rules) ----
B, N, DIM = 2, 2048, 1024
HEADS, DHEAD = 16, 64
T = B * N                      # 4096 tokens
N_CORES = 8
HPC = HEADS // N_CORES         # 2 heads per core
SCALE = DHEAD ** -0.5
BIAS_SCALE = DHEAD ** 0.5
# first distance d belonging to each T5 bucket (verified bit-exact vs the
# float32 reference bucket computation for d in [0, 2048))
BUCKET_STARTS = [0, 1, 2, 3, 4, 5, 6, 7, 8, 9, 10, 11, 12, 13, 14, 15,
                 16, 19, 21, 24, 27, 31, 35, 40, 46, 52, 59, 67, 77, 87, 99, 113]

F16 = mybir.dt.float16
F32 = mybir.dt.float32
F32R = mybir.dt.float32r

_CACHE = {}


def build_program(loop_n=1, phases=(1, 2, 3), p2sub=4):
    """Build + compile the SPMD program. loop_n>1 wraps the body in a timing
    loop (same compute repeated; used only for benchmarking). phases: which
    body phases to emit (benchmarking ablation only)."""
    nc = bacc.Bacc("TRN2", target_bir_lowering=False, debug=False,
                   num_devices=N_CORES)

    xT_d = nc.dram_tensor("xT", (DIM, T), F16, kind="ExternalInput")
    wq_d = nc.dram_tensor("wq", (DIM, 128), F16, kind="ExternalInput")
    wkv_d = nc.dram_tensor("wkv", (DIM, 128), F16, kind="ExternalInput")
    wout_d = nc.dram_tensor("wout", (128, DIM), F16, kind="ExternalInput")
    patt_d = nc.dram_tensor("patt", (128, HPC, 5, 512), F16, kind="ExternalInput")
    identv_d = nc.dram_tensor("identv", (128, 64), F16, kind="ExternalInput")
    y_d = nc.dram_tensor("y", (T, DIM), F16, kind="ExternalOutput")

    TB = T // 512             # 8 token blocks of 512
    TC = T // 128             # 32 token chunks of 128
    NIB = N // 512            # 4 i-blocks per batch
    NJC = N // 128            # 16 j-chunks per batch

    with tile.TileContext(nc) as tc:
        with (
            tc.tile_pool(name="persist", bufs=1) as pers,
            tc.tile_pool(name="xpool", bufs=1) as xpool,
            tc.tile_pool(name="work", bufs=3) as work,
            tc.tile_pool(name="ppool", bufs=3) as ppool,
            tc.tile_pool(name="ps_big", bufs=2, space="PSUM") as ps_big,
            tc.tile_pool(name="ps_acc", bufs=2, space="PSUM") as ps_acc,
        ):
            # ---- persistent SBUF ----
            wq_sb = pers.tile([128, 8, 128], F16)
            wkv_sb = pers.tile([128, 8, 128], F16)
            wout_sb = pers.tile([128, DIM], F16)
            patt_sb = pers.tile([128, HPC, 5, 512], F16)
            identv = pers.tile([128, 64], F16)
            qT = pers.tile([128, T], F32R)       # [2*64 dh, tok]
            kT2 = pers.tile([128, T], F32R)      # kT duplicated on both halves
            kv16 = pers.tile([128, T], F16)      # rows 64:128 = vT (fp16)
            v_ext = pers.tile([128, TC, 65], F16)  # [tok%128, chunk, dv+ones]
            aoT = pers.tile([128, T], F16)       # normalized attn-out (2 heads)
            xsb = xpool.tile([128, 8, T], F16)   # xT slabs, c-chunk major

            for cc in range(8):
                nc.sync.dma_start(wq_sb[:, cc, :], wq_d[cc * 128:(cc + 1) * 128, :])
                nc.sync.dma_start(wkv_sb[:, cc, :], wkv_d[cc * 128:(cc + 1) * 128, :])
            nc.sync.dma_start(wout_sb[:], wout_d[:])
            nc.sync.dma_start(patt_sb[:], patt_d[:])
            nc.sync.dma_start(identv[:], identv_d[:])
            for cc in range(8):
                nc.sync.dma_start(xsb[:, cc, :], xT_d[cc * 128:(cc + 1) * 128, :])
            nc.vector.memset(v_ext[:, :, 64:65], 1.0)

            def body(it):
                # ================= phase 1: projections =================
                for tb in range(TB if 1 in phases else 0):
                    ts = slice(tb * 512, (tb + 1) * 512)
                    q_big = ps_big.tile([128, 1536], F32, name="big")
                    q_ps = q_big[:, 0:512]
                    for cc in range(8):
                        nc.tensor.matmul(q_ps[:], wq_sb[:, cc, :], xsb[:, cc, ts],
                                         start=(cc == 0), stop=(cc == 7))
                    nc.vector.tensor_copy(qT[:, ts], q_ps[:])
                    kv_big = ps_big.tile([128, 1536], F32, name="big")
                    kv_ps = kv_big[:, 0:512]
                    for cc in range(8):
                        nc.tensor.matmul(kv_ps[:], wkv_sb[:, cc, :], xsb[:, cc, ts],
                                         start=(cc == 0), stop=(cc == 7))
                    nc.vector.tensor_copy(kT2[0:64, ts], kv_ps[0:64, :])
                    nc.vector.tensor_copy(kv16[64:128, ts], kv_ps[64:128, :])
                    # v chunks -> natural layout via identity matmul transpose
                    for t4 in range(4):
                        tcix = tb * 4 + t4
                        cs = slice(tb * 512 + t4 * 128, tb * 512 + (t4 + 1) * 128)
                        v_big = ps_big.tile([128, 1536], F32, name="big")
                        v_ps = v_big[:, 0:64]
                        nc.tensor.matmul(v_ps[:], kv16[64:128, cs], identv[64:128, :])
                        nc.vector.tensor_copy(v_ext[:, tcix, 0:64], v_ps[:])
                # duplicate kT into partitions 64:128 (SBUF->SBUF DMA)
                if 1 in phases:
                    nc.sync.dma_start(kT2[64:128, :], kT2[0:64, :])

                # ================= phase 2: attention =================
                # software-pipelined emission: wave n's sim matmuls are
                # emitted before wave n-1's av matmuls so the PE never
                # waits on the exp (ACT) of the wave it is about to consume.
                if 2 in phases:
                    waves = []  # (b, h, ib, jc0, nw)
                    for b in range(B):
                        for h in range(HPC):
                            for ib in range(NIB):
                                J = 4 * (ib + 1)
                                jc = 0
                                while jc < J:
                                    nw = min(3, J - jc)
                                    waves.append((b, h, ib, jc, nw))
                                    jc += nw
                else:
                    waves = []

                state = {}  # (b,h,ib) -> av_ps tile
                pending = []  # [(wave, p_tile)]

                def emit_sim(wv):
                    b, h, ib, jc0, nw = wv
                    hs = slice(h * 64, (h + 1) * 64)
                    i_cols = slice(b * N + ib * 512, b * N + (ib + 1) * 512)
                    sim_ps = ps_big.tile([128, 1536], F32, name="big")
                    for w in range(nw):
                        jcols = slice(b * N + (jc0 + w) * 128,
                                      b * N + (jc0 + w + 1) * 128)
                        nc.tensor.matmul(
                            sim_ps[:, w * 512:(w + 1) * 512],
                            kT2[hs, jcols], qT[hs, i_cols])
                    if p2sub < 2:
                        return None
                    p = ppool.tile([128, 1536], F16, name="p")
                    nc.scalar.activation(
                        p[:, :nw * 512], sim_ps[:, :nw * 512],
                        mybir.ActivationFunctionType.Exp)
                    if p2sub >= 3:
                        for w in range(nw):
                            pidx = (jc0 + w) - (4 * ib - 1)
                            if 0 <= pidx < 5:
                                ps_ = p[:, w * 512:(w + 1) * 512]
                                nc.vector.tensor_mul(
                                    ps_, ps_, patt_sb[:, h, pidx, :])
                    return p

                def emit_av(wv, p):
                    b, h, ib, jc0, nw = wv
                    if p2sub < 3:
                        return
                    hs = slice(h * 64, (h + 1) * 64)
                    i_cols = slice(b * N + ib * 512, b * N + (ib + 1) * 512)
                    J = 4 * (ib + 1)
                    key = (b, h, ib)
                    if key not in state:
                        state[key] = ps_acc.tile([65, 512], F32, name="av_ps")
                    av_ps = state[key]
                    for w in range(nw):
                        nc.tensor.matmul(
                            av_ps[:], v_ext[:, b * NJC + jc0 + w, :],
                            p[:, w * 512:(w + 1) * 512],
                            start=(jc0 + w == 0), stop=(jc0 + w == J - 1))
                    if jc0 + nw == J and p2sub >= 4:
                        # normalize: aoT = av / denom
                        den = work.tile([1, 512], F32, name="den")
                        nc.vector.tensor_copy(den[:], av_ps[64:65, :])
                        rep = work.tile([64, 512], F32, name="rep")
                        nc.gpsimd.partition_broadcast(rep[:], den[:])
                        rec = work.tile([64, 512], F32, name="rec")
                        nc.vector.reciprocal_approx_fast(rec[:], rep[:])
                        nc.vector.tensor_mul(aoT[hs, i_cols], av_ps[0:64, :],
                                             rec[:])
                        del state[key]

                for wv in waves:
                    p = emit_sim(wv)
                    if pending:
                        emit_av(*pending.pop(0))
                    pending.append((wv, p))
                while pending:
                    emit_av(*pending.pop(0))

                # ================= phase 3: out-projection =================
                for tcix in range(TC if 3 in phases else 0):
                    cs = slice(tcix * 128, (tcix + 1) * 128)
                    y_big = ps_big.tile([128, 1536], F32, name="big")
                    y_ps = y_big[:, 0:1024]
                    nc.tensor.matmul(y_ps[:, 0:512], aoT[:, cs], wout_sb[:, 0:512])
                    nc.tensor.matmul(y_ps[:, 512:1024], aoT[:, cs], wout_sb[:, 512:1024])
                    y_sb = ppool.tile([128, 1024], F16, name="y_sb")
                    nc.vector.tensor_copy(y_sb[:, 0:512], y_ps[:, 0:512])
                    nc.scalar.copy(y_sb[:, 512:1024], y_ps[:, 512:1024])
                    nc.sync.dma_start(y_d[cs, :], y_sb[:])

            if loop_n == 1:
                body(0)
            else:
                with tc.For_i(0, loop_n, 1) as it:
                    body(it)

    nc.compile()
    return nc


def make_inputs(x, Wq, Wkv, Wout, rel_emb):
    """Host-side sharding/marshaling -> per-core input maps."""
    xT = np.ascontiguousarray(
        x.reshape(T, DIM).T.astype(np.float32)).astype(np.float16)
    wkv16 = Wkv.astype(np.float16)
    identv = np.concatenate([np.eye(64), np.eye(64)], axis=0).astype(np.float16)

    # bucket table for distances 0..2047 and per-head band patterns
    d_all = np.arange(2048)
    bucket = (np.searchsorted(BUCKET_STARTS, d_all, side="right") - 1).astype(np.int64)
    ii = np.arange(512)[None, :]
    jj = np.arange(128)[:, None]

    in_maps = []
    for c in range(N_CORES):
        wq_c = (Wq[:, c * 128:(c + 1) * 128].astype(np.float64) * SCALE
                ).astype(np.float32).astype(np.float16)
        wout_c = Wout[c * 128:(c + 1) * 128, :].astype(np.float16)
        patt = np.zeros((128, HPC, 5, 512), np.float32)
        for hh in range(HPC):
            h = c * HPC + hh
            delta = (rel_emb[bucket, h] - rel_emb[31, h]).astype(np.float32) * BIAS_SCALE
            e_tab = np.exp(delta)
            for pi in range(5):
                d0 = 128 - 128 * pi
                d = d0 + ii - jj
                patt[:, hh, pi, :] = np.where(
                    d >= 0, e_tab[np.clip(d, 0, 2047)], 0.0)
        in_maps.append({
            "xT": xT,
            "wq": np.ascontiguousarray(wq_c),
            "wkv": wkv16,
            "wout": np.ascontiguousarray(wout_c),
            "patt": patt.astype(np.float16),
            "identv": identv,
        })
    return in_maps


def gather_output(results, bout):
    y = np.zeros((T, DIM), np.float32)
    for c in range(N_CORES):
        y += results[c]["y"].astype(np.float32)
    y += bout.astype(np.float32)[None, :]
    return y.reshape(B, N, DIM)


def kernel(x, Wq, Wkv, Wout, bout, rel_emb):
    x = np.asarray(x); Wq = np.asarray(Wq); Wkv = np.asarray(Wkv)
    Wout = np.asarray(Wout); bout = np.asarray(bout); rel_emb = np.asarray(rel_emb)
    if "nc" not in _CACHE:
        _CACHE["nc"] = build_program()
    nc = _CACHE["nc"]
    in_maps = make_inputs(x, Wq, Wkv, Wout, rel_emb)
    res = run_bass_kernel_spmd(nc, in_maps, list(range(N_CORES)))
    return gather_output(res.results, bout)
